# revision 19
# baseline (speedup 1.0000x reference)
"""Trainium2 Bass kernel for nn_Coords2RMSD (masked Kabsch RMSD loss).

Pure data parallel over 8 NeuronCores (1024 samples each). Per core,
samples are processed in 8 tiles of 128 (partition = sample). Each tile's
X/Y rows stream from HBM once; 17 per-sample reductions (mask-weighted
component sums, sums of squares, and the 3x3 correlation matrix) are
computed with fused multiply-accumulate ops spread across DVE / GPSIMD /
ACT. A closed-form 3x3 eigenvalue epilogue (trig method; polynomial
acos/sin/cos; sqrt via exp(0.5*ln)) turns the reductions into the RMSD.
"""
import math
import numpy as np

P = 128          # partitions (samples per tile)
M = 768          # max atoms
D = 3 * M        # row length
NCORES = 8
T = 8            # tiles per core
S = P * T        # samples per core

_CACHE = {}


def _build(n_tiles):
    import concourse.bacc as bacc
    import concourse.mybir as mybir
    from concourse.tile import TileContext
    from concourse.tile_rust import add_dep_helper

    f32 = mybir.dt.float32
    ALU = mybir.AluOpType
    AF = mybir.ActivationFunctionType

    Tn = n_tiles
    Sn = P * Tn

    nc = bacc.Bacc()
    xd = nc.declare_dram_parameter("x", [Sn, D], f32, isOutput=False)
    yd = nc.declare_dram_parameter("y", [Sn, D], f32, isOutput=False)
    # consts packs [iota3 (D) | nv (Tn) | invn (Tn)] so one DMA covers all
    constsd = nc.declare_dram_parameter("consts", [P, D + 2 * Tn], f32,
                                        isOutput=False)
    outd = nc.declare_dram_parameter("out", [P, Tn], f32, isOutput=True)

    with TileContext(nc) as tc:
        with tc.tile_pool(name="io", bufs=3) as io, \
             tc.tile_pool(name="wk", bufs=2) as wk, \
             tc.tile_pool(name="st", bufs=1) as st:
            consts_t = st.tile([P, D + 2 * Tn], f32)
            nc.sync.dma_start(out=consts_t[:], in_=constsd[:])
            iota3_t = consts_t[:, 0:D]          # atom index repeated 3x
            nv_t = consts_t[:, D:D + Tn]
            invn_t = consts_t[:, D + Tn:D + 2 * Tn]

            # stats accumulators
            mm = st.tile([P, 9 * Tn], f32)    # col (i*3+j)*Tn + t
            sx = st.tile([P, 3 * Tn], f32)    # col i*Tn + t
            sy = st.tile([P, 3 * Tn], f32)
            ssx = st.tile([P, Tn], f32)
            ssy = st.tile([P, Tn], f32)

            # tiny scratch for wait-absorber nops (one per engine)
            pnop = st.tile([P, 1], f32)
            pnop2 = st.tile([P, 1], f32)
            dnop = st.tile([P, 1], f32)

            for t in range(Tn):
                xt = io.tile([P, D], f32, tag="x")
                nc.sync.dma_start(out=xt[:], in_=xd[t * P:(t + 1) * P, :])
                yt = io.tile([P, D], f32, tag="y")
                nc.sync.dma_start(out=yt[:], in_=yd[t * P:(t + 1) * P, :])

                # mask3 = (iota3 < n), full row width, on DVE.
                # Readers: gpsimd only -> its WAR is a single wait.
                mask3 = wk.tile([P, D], f32, tag="mask3")
                nc.vector.tensor_scalar(out=mask3[:], in0=iota3_t,
                                        scalar1=nv_t[:, t:t + 1], scalar2=None,
                                        op0=ALU.is_lt)

                # gpsimd wait-absorbers: each carries at most ONE sync wait so
                # the real (TS-struct-lowered) ops below never need two.
                # Data deps (pnop/pnop2 -> xm[:,0:1] -> full xm) force order.
                iA = nc.gpsimd.tensor_tensor(out=pnop[:], in0=mask3[:, 0:1],
                                             in1=mask3[:, 0:1], op=ALU.mult)
                iB = nc.gpsimd.tensor_tensor(out=pnop2[:], in0=xt[:, 0:1],
                                             in1=xt[:, 0:1], op=ALU.mult)
                xm = wk.tile([P, D], f32, tag="xm")
                # write-absorber: soaks xm's WAR wait on ACT readers
                iC = nc.gpsimd.tensor_tensor(out=xm[:, 0:1], in0=pnop[:],
                                             in1=pnop2[:], op=ALU.mult)

                # masked tensors on gpsimd (true tensor-tensor, no broadcast)
                nc.gpsimd.tensor_tensor(out=xm[:], in0=xt[:], in1=mask3[:],
                                        op=ALU.mult)
                ym = wk.tile([P, D], f32, tag="ym")
                iY = nc.gpsimd.tensor_tensor(out=ym[:], in0=yt[:], in1=mask3[:],
                                             op=ALU.mult)
                add_dep_helper(iY.ins, iC.ins, sync=False,
                               reason="ym WAR covered by xm write-absorber")

                # DVE absorber for the y DMA before the product ops
                iD = nc.vector.tensor_tensor(out=dnop[:], in0=yt[:, 0:1],
                                             in1=yt[:, 0:1], op=ALU.mult)

                # products: m_ij = sum_a xm_i * y_j   (fused accumulate, DVE)
                xmc = xm[:].rearrange("p (a c) -> p c a", c=3)
                ytc = yt[:].rearrange("p (a c) -> p c a", c=3)
                first_prod = None
                for i in range(3):
                    for j in range(3):
                        junk = wk.tile([P, M], f32, tag="junk")
                        col = (i * 3 + j) * Tn + t
                        pr = nc.vector.scalar_tensor_tensor(
                            out=junk[:], in0=xmc[:, i], scalar=1.0,
                            in1=ytc[:, j], op0=ALU.mult, op1=ALU.mult,
                            accum_out=mm[:, col:col + 1])
                        if first_prod is None:
                            first_prod = pr
                            add_dep_helper(pr.ins, iD.ins, sync=False,
                                           reason="y DMA wait absorbed on DVE")

                # X/Y stats on ACT (fused accumulate). ym stats first so the
                # xm write-absorber's ACT wait covers ym's WAR transitively.
                xmcv = xmc
                ymc = ym[:].rearrange("p (a c) -> p c a", c=3)
                sq2 = wk.tile([P, D], f32, tag="sq")
                nc.scalar.activation(out=sq2[:], in_=ym[:], func=AF.Square,
                                     accum_out=ssy[:, t:t + 1])
                for i in range(3):
                    cp2 = wk.tile([P, M], f32, tag="cp")
                    nc.scalar.activation(out=cp2[:], in_=ymc[:, i], func=AF.Copy,
                                         accum_out=sy[:, i * Tn + t:i * Tn + t + 1])
                sq = wk.tile([P, D], f32, tag="sq")
                nc.scalar.activation(out=sq[:], in_=xm[:], func=AF.Square,
                                     accum_out=ssx[:, t:t + 1])
                for i in range(3):
                    cp = wk.tile([P, M], f32, tag="cp")
                    nc.scalar.activation(out=cp[:], in_=xmcv[:, i], func=AF.Copy,
                                         accum_out=sx[:, i * Tn + t:i * Tn + t + 1])

            # ---------------- epilogue (batched over [P, ..., Tn]) ----------
            cnt = [0]

            def new(shape):
                """Allocate a scratch tile and return an AP shaped like `shape`."""
                cnt[0] += 1
                free = int(np.prod(shape[1:]))
                r = st.tile([P, free], f32, tag=f"e{cnt[0]}")
                ap = r[:]
                if len(shape) > 2:
                    names = " ".join(f"d{i}" for i in range(len(shape) - 1))
                    ap = ap.rearrange(f"p ({names}) -> p {names}",
                                      **{f"d{i}": int(shape[1 + i])
                                         for i in range(len(shape) - 1)})
                return ap

            def tt(a, b, op, shape=None):
                r = new(list(shape or a.shape))
                nc.vector.tensor_tensor(out=r, in0=a, in1=b, op=op)
                return r

            def ts(a, s1, op0, s2=None, op1=None):
                r = new(list(a.shape))
                if op1 is None:
                    nc.vector.tensor_scalar(out=r, in0=a, scalar1=s1,
                                            scalar2=None, op0=op0)
                else:
                    nc.vector.tensor_scalar(out=r, in0=a, scalar1=s1,
                                            scalar2=s2, op0=op0, op1=op1)
                return r

            def stt(a, s, b, op0, op1):
                r = new(list(a.shape))
                nc.vector.scalar_tensor_tensor(out=r, in0=a, scalar=s,
                                               in1=b, op0=op0, op1=op1)
                return r

            def act(a, func, scale=1.0, bias=0.0):
                r = new(list(a.shape))
                nc.scalar.activation(out=r, in_=a, func=func,
                                     scale=scale, bias=bias)
                return r

            def recip(a):
                r = new(list(a.shape))
                nc.vector.reciprocal(out=r, in_=a)
                return r

            def sqrt_ln(a, ln_scale=1.0):
                # sqrt(x*ln_scale) = exp(0.5*ln(x*ln_scale)); needs x > 0
                return act(act(a, AF.Ln, scale=ln_scale), AF.Exp, scale=0.5)

            def red_inner(a, n_keep):
                # sum over innermost axis of a multi-dim AP
                r = new([P, n_keep])
                nc.vector.tensor_reduce(out=r, in_=a,
                                        axis=mybir.AxisListType.X, op=ALU.add)
                return r

            def poly_eval(x, coeffs):
                """coeffs [a_n, ..., a_1, a_0] -> a_0 + x*(a_1 + x*(... a_n))"""
                g = ts(x, coeffs[0], ALU.mult)                # x*a_n
                for c in coeffs[1:-1]:
                    g = stt(g, c, x, ALU.add, ALU.mult)       # x*(c + g)
                return ts(g, coeffs[-1], ALU.add)             # + a_0

            mmv = mm[:].rearrange("p (i j t) -> p i j t", i=3, j=3)
            sxv = sx[:].rearrange("p (i t) -> p i t", i=3)
            syv = sy[:].rearrange("p (i t) -> p i t", i=3)
            invn_b3 = invn_t.unsqueeze(1).broadcast_to([P, 3, Tn])

            # R_ij = m_ij - (sx_i * invn) * sy_j
            meanx = tt(sxv, invn_b3, ALU.mult)                       # [P,3,Tn]
            meanx_v = meanx.unsqueeze(2).broadcast_to([P, 3, 3, Tn])
            sy_v = syv.unsqueeze(1).broadcast_to([P, 3, 3, Tn])
            mxsy = tt(meanx_v, sy_v, ALU.mult)                       # [P,3,3,Tn]
            Rv = tt(mmv, mxsy, ALU.subtract)                         # [P,3,3,Tn]

            # e0 = ssx + ssy - (|sx|^2 + |sy|^2) * invn
            sx2 = tt(sxv, sxv, ALU.mult)
            sy2 = tt(syv, syv, ALU.mult)
            nrm = tt(sx2, sy2, ALU.add)
            nrms = red_inner(nrm.rearrange("p i t -> p t i"), Tn)
            ss = tt(ssx[:], ssy[:], ALU.add)
            nrmi = tt(nrms, invn_t, ALU.mult)
            e0 = tt(ss, nrmi, ALU.subtract)                          # [P,Tn]

            # A = R^T R : A_ab = sum_k R_ka * R_kb   (batched outer products)
            Av = new([P, 3, 3, Tn])
            for k in range(3):
                rk = Rv[:, k]                                        # [P,3,Tn]
                rk_a = rk.unsqueeze(2).broadcast_to([P, 3, 3, Tn])
                rk_b = rk.unsqueeze(1).broadcast_to([P, 3, 3, Tn])
                if k == 0:
                    nc.vector.tensor_tensor(out=Av, in0=rk_a, in1=rk_b,
                                            op=ALU.mult)
                else:
                    pk = tt(rk_a, rk_b, ALU.mult)
                    nc.vector.tensor_tensor(out=Av, in0=Av, in1=pk, op=ALU.add)
            Aflat = Av.rearrange("p a b t -> p (a b) t")             # [P,9,Tn]
            Adiag = Aflat[:, ::4]                                    # [P,3,Tn]

            # q = tr(A)/3 ; bdiag = diag(A) - q
            q = ts(red_inner(Adiag.rearrange("p a t -> p t a"), Tn),
                   1.0 / 3.0, ALU.mult)                              # [P,Tn]
            q_b3 = q.unsqueeze(1).broadcast_to([P, 3, Tn])
            bdiag = tt(Adiag, q_b3, ALU.subtract)

            # p2 = sum(bdiag^2) + (sum(A^2) - sum(diag(A)^2))
            asq = tt(Aflat, Aflat, ALU.mult)
            allsq = red_inner(asq.rearrange("p a t -> p t a"), Tn)
            dsq = tt(Adiag, Adiag, ALU.mult)
            dsqs = red_inner(dsq.rearrange("p a t -> p t a"), Tn)
            bsq = tt(bdiag, bdiag, ALU.mult)
            bsqs = red_inner(bsq.rearrange("p a t -> p t a"), Tn)
            offs = tt(allsq, dsqs, ALU.subtract)
            p2 = tt(bsqs, offs, ALU.add)                             # [P,Tn]

            # p = sqrt((p2+eps)/6); invp = 1/p
            p2e = ts(p2, 1e-25, ALU.add)
            p_ = sqrt_ln(p2e, ln_scale=1.0 / 6.0)
            invp = recip(p_)

            # batched determinants of W0=R and W1=B (= A - q I)
            Dw = new([P, 2, 3, 3, Tn])
            nc.vector.tensor_copy(Dw[:, 0], Rv)
            nc.vector.tensor_copy(Dw[:, 1], Av)
            Dw_diag = Dw.rearrange("p w a b t -> p w (a b) t")[:, 1, ::4]
            nc.vector.tensor_tensor(out=Dw_diag, in0=Adiag, in1=q_b3,
                                    op=ALU.subtract)

            def dsl(i, j):
                return Dw[:, :, i, j]                                # [P,2,Tn]

            u1 = tt(dsl(1, 1), dsl(2, 2), ALU.mult)
            u2 = tt(dsl(1, 2), dsl(2, 1), ALU.mult)
            cof0 = tt(dsl(0, 0), tt(u1, u2, ALU.subtract), ALU.mult)
            u3 = tt(dsl(1, 0), dsl(2, 2), ALU.mult)
            u4 = tt(dsl(1, 2), dsl(2, 0), ALU.mult)
            cof1 = tt(dsl(0, 1), tt(u3, u4, ALU.subtract), ALU.mult)
            u5 = tt(dsl(1, 0), dsl(2, 1), ALU.mult)
            u6 = tt(dsl(1, 1), dsl(2, 0), ALU.mult)
            cof2 = tt(dsl(0, 2), tt(u5, u6, ALU.subtract), ALU.mult)
            dets = tt(tt(cof0, cof1, ALU.subtract), cof2, ALU.add)   # [P,2,Tn]
            detR = dets[:, 0]
            detB = dets[:, 1]

            # r = clamp(0.5 * detB * invp^3, -1, 1)
            ip2 = tt(invp, invp, ALU.mult)
            ip3 = tt(ip2, invp, ALU.mult)
            rr = tt(detB, ip3, ALU.mult)
            r_ = ts(rr, 0.5, ALU.mult, 1.0, ALU.min)
            r_ = ts(r_, -1.0, ALU.max)

            # acos(r)/3 via |r| polynomial (A&S 4.4.46) + reflection
            rneg = ts(r_, -1.0, ALU.mult)
            tabs = tt(r_, rneg, ALU.max)
            poly = poly_eval(tabs, [-0.0012624911, 0.0066700901, -0.0170881256,
                                    0.0308918810, -0.0501743046, 0.0889789874,
                                    -0.2145988016, 1.5707963050])
            u_ = ts(tabs, -1.0, ALU.mult, 1.0, ALU.add)
            u_ = ts(u_, 1e-30, ALU.add)
            sq1mt = sqrt_ln(u_)
            # absorber: first DVE read of an ACT product carries the ACT wait
            iE = nc.vector.tensor_tensor(out=dnop[:], in0=sq1mt[:, 0:1],
                                         in1=sq1mt[:, 0:1], op=ALU.mult)
            acos_t = new([P, Tn])
            iF = nc.vector.tensor_tensor(out=acos_t, in0=poly, in1=sq1mt,
                                         op=ALU.mult)
            add_dep_helper(iF.ins, iE.ins, sync=False,
                           reason="ACT wait absorbed before acos mult")
            ind = ts(r_, 0.0, ALU.is_ge)
            sgn = ts(ind, 2.0, ALU.mult, -1.0, ALU.add)
            pio = ts(ind, -math.pi, ALU.mult, math.pi, ALU.add)
            acos_r = tt(tt(acos_t, sgn, ALU.mult), pio, ALU.add)
            phi = ts(acos_r, 1.0 / 3.0, ALU.mult)

            # cos/sin Taylor on [0, pi/3]; cos(phi+2pi/3) = -.5 c - (v3/2) s
            z = tt(phi, phi, ALU.mult)
            cosp = poly_eval(z, [1.0 / 40320, -1.0 / 720, 1.0 / 24, -0.5, 1.0])
            sinp = poly_eval(z, [-1.0 / 5040, 1.0 / 120, -1.0 / 6, 1.0])
            sinp = tt(sinp, phi, ALU.mult)
            halfc = ts(cosp, -0.5, ALU.mult)
            cosp2 = stt(sinp, -math.sqrt(3.0) / 2.0, halfc, ALU.mult, ALU.add)

            twop = ts(p_, 2.0, ALU.mult)
            eigs = new([P, 3, Tn])
            e1t = tt(twop, cosp, ALU.mult)
            nc.vector.tensor_tensor(out=eigs[:, 0], in0=e1t, in1=q, op=ALU.add)
            e3t = tt(twop, cosp2, ALU.mult)
            nc.vector.tensor_tensor(out=eigs[:, 2], in0=e3t, in1=q, op=ALU.add)
            q3 = ts(q, 3.0, ALU.mult)
            e12 = tt(eigs[:, 0], eigs[:, 2], ALU.add)
            nc.vector.tensor_tensor(out=eigs[:, 1], in0=q3, in1=e12,
                                    op=ALU.subtract)

            eig_c = ts(eigs.rearrange("p k t -> p (k t)"), 0.0, ALU.max,
                       1e-30, ALU.add)                                # [P,3Tn]
            sv = sqrt_ln(eig_c).rearrange("p (k t) -> p k t", k=3)

            dind = ts(detR, 0.0, ALU.is_ge)
            dsgn = ts(dind, 2.0, ALU.mult, -1.0, ALU.add)
            s12 = tt(sv[:, 0], sv[:, 1], ALU.add)
            ds3 = tt(dsgn, sv[:, 2], ALU.mult)
            trace = tt(s12, ds3, ALU.add)                             # [P,Tn]

            e_ = stt(trace, -2.0, e0, ALU.mult, ALU.add)
            e_ = ts(e_, 0.0, ALU.max)
            arg = tt(e_, invn_t, ALU.mult)
            arg = ts(arg, 1e-7, ALU.add)
            y0 = sqrt_ln(arg)
            ry = recip(y0)
            ay = tt(arg, ry, ALU.mult)
            outv = ts(tt(y0, ay, ALU.add), 0.5, ALU.mult)

            nc.sync.dma_start(out=outd[:], in_=outv)

    nc.compile()
    return nc


def get_nc(n_tiles=T):
    if n_tiles not in _CACHE:
        _CACHE[n_tiles] = _build(n_tiles)
    return _CACHE[n_tiles]


def _prep_core_inputs(X, Y, nf, n_tiles):
    invn = (np.float32(1.0) / nf).astype(np.float32)
    consts = np.empty((P, D + 2 * n_tiles), np.float32)
    consts[:, 0:D] = np.repeat(np.arange(M, dtype=np.float32), 3)[None, :]
    consts[:, D:D + n_tiles] = nf.reshape(n_tiles, P).T
    consts[:, D + n_tiles:] = invn.reshape(n_tiles, P).T
    return {
        "x": np.ascontiguousarray(X),
        "y": np.ascontiguousarray(Y),
        "consts": consts,
    }


def kernel(input, target, num_atoms):
    from concourse.bass_utils import run_bass_kernel_spmd

    X = np.asarray(input, dtype=np.float32)
    Y = np.asarray(target, dtype=np.float32)
    nf = np.asarray(num_atoms).astype(np.float32)
    B = X.shape[0]
    assert B == NCORES * S, f"unexpected batch {B}"

    nc = get_nc(T)
    in_maps = []
    for c in range(NCORES):
        sl = slice(c * S, (c + 1) * S)
        in_maps.append(_prep_core_inputs(X[sl], Y[sl], nf[sl], T))
    res = run_bass_kernel_spmd(nc, in_maps, list(range(NCORES))).results
    out = np.empty((NCORES, S), np.float32)
    for c in range(NCORES):
        out[c] = res[c]["out"].T.reshape(S)   # out[p,t] -> sample t*P+p
    return out.reshape(B)


# revision 21
# speedup vs baseline: 1.1729x; 1.1729x over previous
"""Trainium2 Bass kernel for nn_Coords2RMSD (masked Kabsch RMSD loss).

Pure data parallel over 8 NeuronCores (1024 samples each). Inputs are
pre-planarized on the host (each row [x1(768)|x2(768)|x3(768)]) so every
device op is contiguous. Per core, samples are processed in 8 tiles of
128 (partition = sample). Each tile's X/Y rows stream from HBM once; 17
per-sample reductions (mask-weighted component sums, sums of squares, and
the 3x3 correlation matrix) are computed with fused multiply-accumulate
ops spread across DVE / GPSIMD / ACT. A closed-form 3x3 eigenvalue
epilogue (trig method; polynomial acos/sin/cos; sqrt via exp(0.5*ln))
turns the reductions into the RMSD.
"""
import math
import numpy as np

P = 128          # partitions (samples per tile)
M = 768          # max atoms
D = 3 * M        # row length
NCORES = 8
T = 8            # tiles per core
S = P * T        # samples per core
YM_DVE = D // 2  # elements of ym built on DVE (rest on gpsimd)

_CACHE = {}


def _build(n_tiles):
    import concourse.bacc as bacc
    import concourse.mybir as mybir
    from concourse.tile import TileContext
    from concourse.hw_specs import get_activation_tables

    f32 = mybir.dt.float32
    ALU = mybir.AluOpType
    AF = mybir.ActivationFunctionType

    Tn = n_tiles
    Sn = P * Tn

    nc = bacc.Bacc()
    xd = nc.declare_dram_parameter("x", [Sn, D], f32, isOutput=False)
    yd = nc.declare_dram_parameter("y", [Sn, D], f32, isOutput=False)
    # consts packs [iota_planar (D) | nv (Tn) | invn (Tn)]
    constsd = nc.declare_dram_parameter("consts", [P, D + 2 * Tn], f32,
                                        isOutput=False)
    outd = nc.declare_dram_parameter("out", [P, Tn], f32, isOutput=True)

    with TileContext(nc) as tc:
        with tc.tile_pool(name="io", bufs=3) as io, \
             tc.tile_pool(name="wk", bufs=2) as wk, \
             tc.tile_pool(name="st", bufs=1) as st:
            consts_t = st.tile([P, D + 2 * Tn], f32)
            nc.sync.dma_start(out=consts_t[:], in_=constsd[:])
            iota_t = consts_t[:, 0:D]           # planar atom index (x3)
            nv_t = consts_t[:, D:D + Tn]
            invn_t = consts_t[:, D + Tn:D + 2 * Tn]

            # stats accumulators
            mm = st.tile([P, 9 * Tn], f32)    # col (i*3+j)*Tn + t
            sx = st.tile([P, 3 * Tn], f32)    # col i*Tn + t
            sy = st.tile([P, 3 * Tn], f32)
            ssx = st.tile([P, Tn], f32)
            ssy = st.tile([P, Tn], f32)

            for t in range(Tn):
                xt = io.tile([P, D], f32, tag="x")
                nc.sync.dma_start(out=xt[:], in_=xd[t * P:(t + 1) * P, :])
                yt = io.tile([P, D], f32, tag="y")
                nc.sync.dma_start(out=yt[:], in_=yd[t * P:(t + 1) * P, :])

                # mask = (iota < n), planar, on DVE (single-src 2x mode)
                mask3 = wk.tile([P, D], f32, tag="mask3")
                nc.vector.tensor_scalar(out=mask3[:], in0=iota_t,
                                        scalar1=nv_t[:, t:t + 1], scalar2=None,
                                        op0=ALU.is_lt)

                # masked tensors: xm on gpsimd; ym split gpsimd/DVE
                xm = wk.tile([P, D], f32, tag="xm")
                nc.gpsimd.tensor_tensor(out=xm[:], in0=xt[:], in1=mask3[:],
                                        op=ALU.mult)
                ym = wk.tile([P, D], f32, tag="ym")
                h = D - YM_DVE
                nc.gpsimd.tensor_tensor(out=ym[:, :h], in0=yt[:, :h],
                                        in1=mask3[:, :h], op=ALU.mult)
                nc.vector.tensor_tensor(out=ym[:, h:], in0=yt[:, h:],
                                        in1=mask3[:, h:], op=ALU.mult)

                # products: m_ij = sum_a xm_i * y_j (fused accumulate, DVE)
                for i in range(3):
                    for j in range(3):
                        junk = wk.tile([P, M], f32, tag="junk")
                        col = (i * 3 + j) * Tn + t
                        nc.vector.scalar_tensor_tensor(
                            out=junk[:], in0=xm[:, i * M:(i + 1) * M],
                            scalar=1.0, in1=yt[:, j * M:(j + 1) * M],
                            op0=ALU.mult, op1=ALU.mult,
                            accum_out=mm[:, col:col + 1])

                # X/Y stats on ACT (fused accumulate)
                sq2 = wk.tile([P, D], f32, tag="sq")
                nc.scalar.activation(out=sq2[:], in_=ym[:], func=AF.Square,
                                     accum_out=ssy[:, t:t + 1])
                for i in range(3):
                    cp2 = wk.tile([P, M], f32, tag="cp")
                    nc.scalar.activation(out=cp2[:], in_=ym[:, i * M:(i + 1) * M],
                                         func=AF.Copy,
                                         accum_out=sy[:, i * Tn + t:i * Tn + t + 1])
                sq = wk.tile([P, D], f32, tag="sq")
                nc.scalar.activation(out=sq[:], in_=xm[:], func=AF.Square,
                                     accum_out=ssx[:, t:t + 1])
                for i in range(3):
                    cp = wk.tile([P, M], f32, tag="cp")
                    nc.scalar.activation(out=cp[:], in_=xm[:, i * M:(i + 1) * M],
                                         func=AF.Copy,
                                         accum_out=sx[:, i * Tn + t:i * Tn + t + 1])

            # ---------------- epilogue (batched over [P, ..., Tn]) ----------
            cnt = [0]

            def new(shape):
                """Allocate a scratch tile; return an AP shaped like `shape`."""
                cnt[0] += 1
                free = int(np.prod(shape[1:]))
                r = st.tile([P, free], f32, tag=f"e{cnt[0]}")
                ap = r[:]
                if len(shape) > 2:
                    names = " ".join(f"d{i}" for i in range(len(shape) - 1))
                    ap = ap.rearrange(f"p ({names}) -> p {names}",
                                      **{f"d{i}": int(shape[1 + i])
                                         for i in range(len(shape) - 1)})
                return ap

            def tt(a, b, op, shape=None):
                r = new(list(shape or a.shape))
                nc.vector.tensor_tensor(out=r, in0=a, in1=b, op=op)
                return r

            def ts(a, s1, op0, s2=None, op1=None):
                r = new(list(a.shape))
                if op1 is None:
                    nc.vector.tensor_scalar(out=r, in0=a, scalar1=s1,
                                            scalar2=None, op0=op0)
                else:
                    nc.vector.tensor_scalar(out=r, in0=a, scalar1=s1,
                                            scalar2=s2, op0=op0, op1=op1)
                return r

            def stt(a, s, b, op0, op1):
                r = new(list(a.shape))
                nc.vector.scalar_tensor_tensor(out=r, in0=a, scalar=s,
                                               in1=b, op0=op0, op1=op1)
                return r

            def act(a, func, scale=1.0, bias=0.0):
                r = new(list(a.shape))
                nc.scalar.activation(out=r, in_=a, func=func,
                                     scale=scale, bias=bias)
                return r

            def recip(a):
                r = new(list(a.shape))
                nc.vector.reciprocal(out=r, in_=a)
                return r

            def red_inner(a, n_keep):
                r = new([P, n_keep])
                nc.vector.tensor_reduce(out=r, in_=a,
                                        axis=mybir.AxisListType.X, op=ALU.add)
                return r

            def poly_eval(x, coeffs):
                """coeffs [a_n..a_1, a_0] -> a_0 + x*(a_1 + x*(...a_n))"""
                g = ts(x, coeffs[0], ALU.mult)
                for c in coeffs[1:-1]:
                    g = stt(g, c, x, ALU.add, ALU.mult)
                return ts(g, coeffs[-1], ALU.add)

            mmv = mm[:].rearrange("p (i j t) -> p i j t", i=3, j=3)
            sxv = sx[:].rearrange("p (i t) -> p i t", i=3)
            syv = sy[:].rearrange("p (i t) -> p i t", i=3)
            invn_b3 = invn_t.unsqueeze(1).broadcast_to([P, 3, Tn])

            # R_ij = m_ij - (sx_i * invn) * sy_j
            meanx = tt(sxv, invn_b3, ALU.mult)                       # [P,3,Tn]
            meanx_v = meanx.unsqueeze(2).broadcast_to([P, 3, 3, Tn])
            sy_v = syv.unsqueeze(1).broadcast_to([P, 3, 3, Tn])
            mxsy = tt(meanx_v, sy_v, ALU.mult)
            Rv = tt(mmv, mxsy, ALU.subtract)                         # [P,3,3,Tn]

            # e0 = ssx + ssy - (|sx|^2 + |sy|^2) * invn
            sx2 = tt(sxv, sxv, ALU.mult)
            sy2 = tt(syv, syv, ALU.mult)
            nrm = tt(sx2, sy2, ALU.add)
            nrms = red_inner(nrm.rearrange("p i t -> p t i"), Tn)
            ss = tt(ssx[:], ssy[:], ALU.add)
            nrmi = tt(nrms, invn_t, ALU.mult)
            e0 = tt(ss, nrmi, ALU.subtract)                          # [P,Tn]

            # A = R^T R (batched outer products over k)
            Av = new([P, 3, 3, Tn])
            for k in range(3):
                rk = Rv[:, k]
                rk_a = rk.unsqueeze(2).broadcast_to([P, 3, 3, Tn])
                rk_b = rk.unsqueeze(1).broadcast_to([P, 3, 3, Tn])
                if k == 0:
                    nc.vector.tensor_tensor(out=Av, in0=rk_a, in1=rk_b,
                                            op=ALU.mult)
                else:
                    pk = tt(rk_a, rk_b, ALU.mult)
                    nc.vector.tensor_tensor(out=Av, in0=Av, in1=pk, op=ALU.add)
            Aflat = Av.rearrange("p a b t -> p (a b) t")
            Adiag = Aflat[:, ::4]                                    # [P,3,Tn]

            q = ts(red_inner(Adiag.rearrange("p a t -> p t a"), Tn),
                   1.0 / 3.0, ALU.mult)                              # [P,Tn]
            q_b3 = q.unsqueeze(1).broadcast_to([P, 3, Tn])
            bdiag = tt(Adiag, q_b3, ALU.subtract)

            # p2 = sum(bdiag^2) + (sum(A^2) - sum(diag(A)^2))
            asq = tt(Aflat, Aflat, ALU.mult)
            allsq = red_inner(asq.rearrange("p a t -> p t a"), Tn)
            dsq = tt(Adiag, Adiag, ALU.mult)
            dsqs = red_inner(dsq.rearrange("p a t -> p t a"), Tn)
            bsq = tt(bdiag, bdiag, ALU.mult)
            bsqs = red_inner(bsq.rearrange("p a t -> p t a"), Tn)
            offs = tt(allsq, dsqs, ALU.subtract)
            p2 = tt(bsqs, offs, ALU.add)                             # [P,Tn]

            # log-space: p = (p2/6)^0.5 and invp^3 = (p2/6)^-1.5
            p2e = ts(p2, 1e-10, ALU.add)
            lnp2 = act(p2e, AF.Ln, scale=1.0 / 6.0)
            p_ = act(lnp2, AF.Exp, scale=0.5)
            ip3 = act(lnp2, AF.Exp, scale=-1.5)

            # batched determinants of W0=R and W1=B (= A - q I)
            Dw = new([P, 2, 3, 3, Tn])
            nc.vector.tensor_copy(Dw[:, 0], Rv)
            nc.vector.tensor_copy(Dw[:, 1], Av)
            Dw_diag = Dw.rearrange("p w a b t -> p w (a b) t")[:, 1, ::4]
            nc.vector.tensor_tensor(out=Dw_diag, in0=Adiag, in1=q_b3,
                                    op=ALU.subtract)

            def dsl(i, j):
                return Dw[:, :, i, j]                                # [P,2,Tn]

            u1 = tt(dsl(1, 1), dsl(2, 2), ALU.mult)
            u2 = tt(dsl(1, 2), dsl(2, 1), ALU.mult)
            cof0 = tt(dsl(0, 0), tt(u1, u2, ALU.subtract), ALU.mult)
            u3 = tt(dsl(1, 0), dsl(2, 2), ALU.mult)
            u4 = tt(dsl(1, 2), dsl(2, 0), ALU.mult)
            cof1 = tt(dsl(0, 1), tt(u3, u4, ALU.subtract), ALU.mult)
            u5 = tt(dsl(1, 0), dsl(2, 1), ALU.mult)
            u6 = tt(dsl(1, 1), dsl(2, 0), ALU.mult)
            cof2 = tt(dsl(0, 2), tt(u5, u6, ALU.subtract), ALU.mult)
            dets = tt(tt(cof0, cof1, ALU.subtract), cof2, ALU.add)   # [P,2,Tn]
            detR = dets[:, 0]
            detB = dets[:, 1]

            # r = clamp(0.5 * detB * invp^3, -1, 1)
            rr = tt(detB, ip3, ALU.mult)
            r_ = ts(rr, 0.5, ALU.mult, 1.0, ALU.min)
            r_ = ts(r_, -1.0, ALU.max)

            # acos(r)/3 via |r| polynomial (A&S 4.4.46) + reflection
            rneg = ts(r_, -1.0, ALU.mult)
            tabs = tt(r_, rneg, ALU.max)
            poly = poly_eval(tabs, [-0.0012624911, 0.0066700901, -0.0170881256,
                                    0.0308918810, -0.0501743046, 0.0889789874,
                                    -0.2145988016, 1.5707963050])
            u_ = ts(tabs, -1.0, ALU.mult, 1.0, ALU.add)
            u_ = ts(u_, 1e-30, ALU.add)
            sq1mt = act(act(u_, AF.Ln), AF.Exp, scale=0.5)
            acos_t = tt(poly, sq1mt, ALU.mult)
            ind = ts(r_, 0.0, ALU.is_ge)
            sgn = ts(ind, 2.0, ALU.mult, -1.0, ALU.add)
            pio = ts(ind, -math.pi, ALU.mult, math.pi, ALU.add)
            acos_r = tt(tt(acos_t, sgn, ALU.mult), pio, ALU.add)
            phi = ts(acos_r, 1.0 / 3.0, ALU.mult)

            # cos/sin Taylor on [0, pi/3]; cos(phi+2pi/3) = -.5 c - (v3/2) s
            z = tt(phi, phi, ALU.mult)
            cosp = poly_eval(z, [1.0 / 40320, -1.0 / 720, 1.0 / 24, -0.5, 1.0])
            sinp = poly_eval(z, [-1.0 / 5040, 1.0 / 120, -1.0 / 6, 1.0])
            sinp = tt(sinp, phi, ALU.mult)
            halfc = ts(cosp, -0.5, ALU.mult)
            cosp2 = stt(sinp, -math.sqrt(3.0) / 2.0, halfc, ALU.mult, ALU.add)

            twop = ts(p_, 2.0, ALU.mult)
            eigs = new([P, 3, Tn])
            e1t = tt(twop, cosp, ALU.mult)
            nc.vector.tensor_tensor(out=eigs[:, 0], in0=e1t, in1=q, op=ALU.add)
            e3t = tt(twop, cosp2, ALU.mult)
            nc.vector.tensor_tensor(out=eigs[:, 2], in0=e3t, in1=q, op=ALU.add)
            q3 = ts(q, 3.0, ALU.mult)
            e12 = tt(eigs[:, 0], eigs[:, 2], ALU.add)
            nc.vector.tensor_tensor(out=eigs[:, 1], in0=q3, in1=e12,
                                    op=ALU.subtract)

            eig_c = ts(eigs.rearrange("p k t -> p (k t)"), 0.0, ALU.max,
                       1e-30, ALU.add)                                # [P,3Tn]
            sv = act(act(eig_c, AF.Ln), AF.Exp, scale=0.5)
            sv = sv.rearrange("p (k t) -> p k t", k=3)

            dind = ts(detR, 0.0, ALU.is_ge)
            dsgn = ts(dind, 2.0, ALU.mult, -1.0, ALU.add)
            s12 = tt(sv[:, 0], sv[:, 1], ALU.add)
            ds3 = tt(dsgn, sv[:, 2], ALU.mult)
            trace = tt(s12, ds3, ALU.add)                             # [P,Tn]

            e_ = stt(trace, -2.0, e0, ALU.mult, ALU.add)
            e_ = ts(e_, 0.0, ALU.max)
            arg = tt(e_, invn_t, ALU.mult)
            arg = ts(arg, 1e-7, ALU.add)
            y0 = act(act(arg, AF.Ln), AF.Exp, scale=0.5)
            ry = recip(y0)
            ay = tt(arg, ry, ALU.mult)
            outv = ts(tt(y0, ay, ALU.add), 0.5, ALU.mult)

            nc.sync.dma_start(out=outd[:], in_=outv)

    nc.compile()

    # collapse redundant ACT table loads: every function we use (Copy,
    # Square, Ln, Exp) lives in natural_log_exp_and_others, but the
    # chooser ping-pongs between smaller sets. Retarget all loads to the
    # combined set and drop the now-redundant ones (keeping any that
    # carry sync commands).
    tables = list(get_activation_tables(nc.m.arch).keys())
    target = tables.index("natural_log_exp_and_others")
    for blk in nc.main_func.blocks:
        seen = False
        drop = []
        for inst in list(blk.instructions):
            if isinstance(inst, mybir.InstLoadActFuncSet):
                inst.act_func_set_id = target
                si = inst.sync_info
                has_sync = si is not None and (si.on_wait or si.on_update)
                if seen and not has_sync:
                    drop.append(inst)
                    continue
                seen = True
        for inst in drop:
            blk.instructions.remove(inst)
    return nc


def get_nc(n_tiles=T):
    if n_tiles not in _CACHE:
        _CACHE[n_tiles] = _build(n_tiles)
    return _CACHE[n_tiles]


def _planarize(A):
    """[B, (a c)] -> [B, (c a)] rows."""
    B = A.shape[0]
    return np.ascontiguousarray(
        A.reshape(B, M, 3).transpose(0, 2, 1).reshape(B, D))


def _prep_core_inputs(X, Y, nf, n_tiles):
    invn = (np.float32(1.0) / nf).astype(np.float32)
    consts = np.empty((P, D + 2 * n_tiles), np.float32)
    consts[:, 0:D] = np.tile(np.arange(M, dtype=np.float32), 3)[None, :]
    consts[:, D:D + n_tiles] = nf.reshape(n_tiles, P).T
    consts[:, D + n_tiles:] = invn.reshape(n_tiles, P).T
    return {
        "x": _planarize(X),
        "y": _planarize(Y),
        "consts": consts,
    }


def kernel(input, target, num_atoms):
    from concourse.bass_utils import run_bass_kernel_spmd

    X = np.asarray(input, dtype=np.float32)
    Y = np.asarray(target, dtype=np.float32)
    nf = np.asarray(num_atoms).astype(np.float32)
    B = X.shape[0]
    assert B == NCORES * S, f"unexpected batch {B}"

    nc = get_nc(T)
    in_maps = []
    for c in range(NCORES):
        sl = slice(c * S, (c + 1) * S)
        in_maps.append(_prep_core_inputs(X[sl], Y[sl], nf[sl], T))
    res = run_bass_kernel_spmd(nc, in_maps, list(range(NCORES))).results
    out = np.empty((NCORES, S), np.float32)
    for c in range(NCORES):
        out[c] = res[c]["out"].T.reshape(S)   # out[p,t] -> sample t*P+p
    return out.reshape(B)
